# revision 83
# baseline (speedup 1.0000x reference)
"""Trainium2 Bass kernel: causal multi-head attention with an extra time-mixing
matrix D (attn = D @ softmax(mask(Q K^T / sqrt(e))) @ V, concat heads, out proj).

Shapes (hardcoded): B=4, T=2048, d=1024, H=16, e=64, fp32 in/out.
Sharding over 8 NeuronCores: data-parallel over batch (4) x tensor-parallel over
heads (2 groups of 8). Each core computes its batch/head-group partial
y_part = concat(attn_heads) @ Wo_part^T; host sums the 2 partials per batch and
adds bo.

All matmul data is bf16 (host-converted). Matmul cost on PE is out-free-size
cycles regardless of contraction depth, so PV runs with 65-wide moving ([V|1]
per head; the ones column accumulates softmax row sums in the same matmul)
against stationary exp-score chunks, and scores skip fully-masked 128x128
blocks. exp runs on the scalar engine straight out of PSUM with causal-ragged
access patterns; only diagonal 128x128 blocks need the mask multiply.

Schedule: i-outer (query blocks), p-inner (head pairs). The exp stream keeps
the scalar engine near-saturated during attention, so all remaining PE work is
fed in as fillers between score chunks: V projection for this block, Q/K
projections for the next block, and D@PV accumulation for half the output
columns (a2 psum banks held across the whole loop). The tail finishes D@PV for
the other half plus the output projection. All DMAs are batched multi-dim
transfers (HWDGE charges a fixed ~625ns per DMA instruction).
"""

import sys

for _p in ("/opt/trn_rl_repo", "/root/.axon_site/_ro/trn_rl_repo"):
    if _p not in sys.path:
        sys.path.append(_p)

from contextlib import ExitStack

import ml_dtypes
import numpy as np

import concourse.bass as bass  # noqa: F401  (AP helpers)
import concourse.tile as tile
from concourse import bacc, mybir
from concourse.bass_utils import run_bass_kernel_spmd

dt = mybir.dt
BF = ml_dtypes.bfloat16

B, T, D, H, E = 4, 2048, 1024, 16, 64
HG = 8          # heads per core (tensor-parallel group)
COEF = 1.0 / E ** 0.5
P = 128         # partitions
SCL = 32.0      # Wq/Wk/Wv/Wo host-side scale: keeps fp8 hi/lo splits in
                # normal range; exp scale and the final Copy compensate
SCLD = 8.0      # D host-side scale (a2 psum stays under fp8 max 448)
NTC = T // P    # 16 time chunks
PRE3 = 8        # exp chunks of step (3,0) precomputed in block 2's slack
ND = D // P     # 8 contraction chunks (d)

_CACHED_NC = None


def _build_nc():
    """Build + compile the single-core program (same NEFF on all 8 cores)."""
    nc = bacc.Bacc("TRN2", target_bir_lowering=False, debug=False)
    f32, bf16 = dt.float32, dt.bfloat16
    Exp = mybir.ActivationFunctionType.Exp
    Copy = mybir.ActivationFunctionType.Copy
    mult = mybir.AluOpType.mult
    add = mybir.AluOpType.add

    f8 = dt.float8e4
    DR = mybir.MatmulPerfMode.DoubleRow
    x8d, w8d = {}, {}
    for nm in ("xq", "xk", "xv"):
        for hl in "hl":
            x8d[nm + hl] = nc.dram_tensor(
                nm + hl, [D, T], f8, kind="ExternalInput").ap()
    for nm in ("wq", "wk", "wv"):
        for hl in "hl":
            w8d[nm + hl] = nc.dram_tensor(
                nm + hl, [D, 512], f8, kind="ExternalInput").ap()
    for nm in ("wo", "dt"):
        for hl in "hl":
            w8d[nm + hl] = nc.dram_tensor(
                nm + hl, [512, D] if nm == "wo" else [T, T], f8,
                kind="ExternalInput").ap()
    msk = nc.dram_tensor("msk", [P, 256], bf16, kind="ExternalInput").ap()
    bqc = nc.dram_tensor("bqc", [P, 4], f32, kind="ExternalInput").ap()
    bkc = nc.dram_tensor("bkc", [P, 4], f32, kind="ExternalInput").ap()
    bvr = nc.dram_tensor("bvr", [P, 512], bf16, kind="ExternalInput").ap()
    y = nc.dram_tensor("y", [T, D], bf16, kind="ExternalOutput").ap()

    with tile.TileContext(nc) as tc, ExitStack() as ctx:
        # ---- constants + persistent tiles ----------------------------------
        consts = ctx.enter_context(tc.tile_pool(name="consts", bufs=1))
        mskt = consts.tile([P, 256], bf16, tag="mskt")
        bqt = consts.tile([P, 4], f32, tag="bqt")
        bkt = consts.tile([P, 4], f32, tag="bkt")
        bvt = consts.tile([P, 512], bf16, tag="bvt")
        warm = consts.tile([P, 1], f32, tag="warm")

        persist = ctx.enter_context(tc.tile_pool(name="persist", bufs=1))
        qt = [persist.tile([P, T], bf16, tag=f"qt{p}", name=f"qt{p}") for p in range(4)]
        kt = [persist.tile([P, T], bf16, tag=f"kt{p}", name=f"kt{p}") for p in range(4)]
        # vt: per t-chunk [V_h | 1] blocks: col 65*h+e = V, col 65*h+64 = 1.0
        vt = [persist.tile([P, HG * (E + 1)], bf16, tag=f"vt{t}", name=f"vt{t}")
              for t in range(NTC)]
        # pvg: normalized softmax(S) @ V staging (heads side by side); only
        # feeds the fp8 hi/lo requantization now, so a ring of 8 suffices
        pvgr = [persist.tile([P, 512], bf16, tag=f"pvg{t}", name=f"pvg{t}")
                for t in range(8)]
        pvg = {j: pvgr[j % 8] for j in range(NTC)}
        # fp8 hi/lo requantization of pvg, packed in t-chunk PAIRS for the
        # DoubleRow D@PV in the tail; filled by DVE as pvg completes
        p8 = {hl: [persist.tile([P, 1024], dt.float8e4, tag=f"p8{hl}{tp}",
                                name=f"p8{hl}{tp}")
                   for tp in range(NTC // 2)] for hl in "hl"}

        # a2 accumulators for (qb=0, pc=0/1) — held across the attention loop
        # so part of D@PV streams in as attention fillers (2 psum banks).
        a2ps = ctx.enter_context(tc.tile_pool(name="a2ps", bufs=1, space="PSUM"))
        a2w = [a2ps.tile([P, 512], f32, tag=f"a2w{q}", name=f"a2w{q}")
               for q in range(2)]

        with tc.tile_pool(name="wpool", bufs=1) as wpool, \
             tc.tile_pool(name="xpool", bufs=1) as xpool, \
             tc.tile_pool(name="dtw", bufs=1) as dtwp, \
             tc.tile_pool(name="upool", bufs=1) as upool, \
             tc.tile_pool(name="rpool", bufs=4) as rpool, \
             tc.tile_pool(name="sps", bufs=2, space="PSUM") as sps, \
             tc.tile_pool(name="pvps", bufs=1, space="PSUM") as pvps, \
             tc.tile_pool(name="pjps", bufs=1, space="PSUM") as pjps:

            def load_wb(nm, split=False):
                tiles = []
                for hl in "hl":
                    wdram = w8d[nm + hl]
                    wb = wpool.tile([P, ND * 512], f8, tag=nm + hl, name=nm + hl)
                    nd = 2 if split else 1
                    hd = ND // nd
                    for s in range(nd):
                        nc.sync.dma_start(
                            wb[:, 512 * hd * s:512 * hd * (s + 1)].rearrange(
                                "p (d q) -> p d q", q=512),
                            wdram[P * hd * s:P * hd * (s + 1), :].rearrange(
                                "(d p) q -> p d q", p=P))
                    tiles.append(wb)
                    split = False
                return tiles  # [hi tile, lo tile]

            def drslice(tile_, kk, m0=None):
                """[128, 2 k-tiles, m] DoubleRow operand slice for super-chunk kk."""
                a = tile_[:, 1024 * kk:1024 * (kk + 1)].rearrange(
                    "p (j m) -> p j m", m=512)
                return a if m0 is None else a[:, :, m0:m0 + P]

            # wq + xq(0) load first: the prologue's first matmul needs them
            wq = load_wb("wq", split=True)

            # proj psum ring: one pjps bank, plus the two a2w banks which
            # are free until the D@PV fillers start accumulating at i=3
            pj0 = pjps.tile([P, 512], f32, tag="pj", name="pj")
            pjring = [[pj0, a2w[0], a2w[1]], [pj0]]
            pjslot = [0]

            # two generations of exp-score tiles (step parity) so PV of step
            # s-1 can interleave with scores of step s
            ut2 = [[upool.tile([P, 1024], bf16, tag=f"u{g}_{c}", name=f"u{g}_{c}")
                    for c in range(NTC)] for g in range(2)]
            mskr = mskt[:].rearrange("p (s c) -> p s c", c=P)
            pvt = pvps.tile([P, 390], f32, tag="pvt", name="pvt")  # 3 slots x 130
            pvslot = [0]

            def uchunk(par, c, h, jj):
                return ut2[par][c][:, 512 * h + P * jj:512 * h + P * (jj + 1)]

            xcur = {}

            def issue_x1(kind, tb, split=False):
                pair = []
                for hl in "hl":
                    dram = x8d[kind + hl]
                    xb = xpool.tile([P, ND * 512], f8, tag=kind + hl,
                                    name=kind + hl,
                                    bufs=2 if kind == "xk" else 1)
                    nd = 2 if split else 1
                    hd = ND // nd
                    src = dram[:, 512 * tb:512 * (tb + 1)]
                    for s in range(nd):
                        nc.sync.dma_start(
                            xb[:, 512 * hd * s:512 * hd * (s + 1)].rearrange(
                                "p (d q) -> p d q", q=512),
                            src[P * hd * s:P * hd * (s + 1), :].rearrange(
                                "(d p) q -> p d q", p=P))
                    pair.append(xb)
                    split = False
                xcur[(kind, tb)] = pair

            def issue_x(tb):
                for kind in ("xq", "xk", "xv"):
                    issue_x1(kind, tb)

            issue_x1("xq", 0, split=True)
            nc.sync.dma_start(bqt[:], bqc[:])
            wk = load_wb("wk")
            issue_x1("xk", 0)
            nc.sync.dma_start(bkt[:], bkc[:])
            wv = load_wb("wv")
            issue_x1("xv", 0)
            nc.sync.dma_start(bvt[:], bvr[:])
            nc.sync.dma_start(mskt[:], msk[:])
            nc.vector.memset(warm[:], 0.0)
            nc.scalar.activation(warm[:], warm[:], Exp)  # preload exp table

            dt8w = {}

            def issue_dt8w():
                for hl in "hl":
                    b = dtwp.tile([P, 12 * 512], f8, tag="dt8" + hl,
                                  name="dt8w")
                    nc.sync.dma_start(
                        b[:].rearrange("p (t q) -> p t q", q=512),
                        w8d["dt" + hl][0:1536, 0:512].rearrange(
                            "(t p) q -> p t q", p=P))
                    dt8w[hl] = b

            # ---- filler units (each emits ~0.8-2us of PE work) -------------
            # Projections run as fp8 DoubleRow 3-pass (hi*hi + hi*lo + lo*hi):
            # 12 matmuls of 256 k-depth each, 0.5 cycles/row.
            PASSES = ((0, 0), (0, 1), (1, 0))  # (stationary hi/lo, moving hi/lo)

            def proj_units(p, tb, which, proto=False):
                """Q or K projection of q-block tb for head pair p."""
                wt, bt, dest, kind = {
                    "q": (wq, bqt, qt, "xq"), "k": (wk, bkt, kt, "xk")}[which]

                def go(half, holder=[None]):
                    xp = xcur[(kind, tb)]
                    if half == 0:
                        if proto:  # borrow the idle scores ring (4 banks)
                            sp_ = sps.tile([P, 1024], f32, tag="sp", name="sp")
                            holder[0] = sp_[:, 0:512]
                        else:
                            # runs at block tb-1 (Q) / tb (K); a2w banks are
                            # free until the D@PV fillers start at block 3
                            blk = tb - 1 if which == "q" else tb
                            ring = pjring[0 if blk < 3 else 1]
                            holder[0] = ring[pjslot[0] % len(ring)]
                            pjslot[0] += 1
                    ps = holder[0]
                    for idx in range(6 * half, 6 * half + 6):
                        pw, px = PASSES[idx // 4]
                        kk = idx % 4
                        nc.tensor.matmul(
                            ps[:], drslice(wt[pw], kk, P * p),
                            drslice(xp[px], kk),
                            start=(idx == 0), stop=(idx == 11), perf_mode=DR)
                    if half == 1:
                        nc.vector.tensor_scalar(
                            dest[p][:, 512 * tb:512 * (tb + 1)], ps[:],
                            bt[:, p:p + 1], None, op0=add)
                return [lambda: go(0), lambda: go(1)]

            def v_proj_units(tb, tcc):
                """V projection for t-chunk 4*tb+tcc."""
                units = []

                def go(half, holder=[None]):
                    xp = xcur[("xv", tb)]
                    if half == 0:
                        ring = pjring[0 if tb < 3 else 1]
                        holder[0] = ring[pjslot[0] % len(ring)]
                        pjslot[0] += 1
                    ps = holder[0]
                    for idx in range(6 * half, 6 * half + 6):
                        px, pw = PASSES[idx // 4]
                        kk = idx % 4
                        nc.tensor.matmul(
                            ps[:], drslice(xp[px], kk, P * tcc),
                            drslice(wv[pw], kk),
                            start=(idx == 0), stop=(idx == 11), perf_mode=DR)
                    if half == 1:
                        t = 4 * tb + tcc
                        vtr = vt[t][:].rearrange("p (h c) -> p h c", c=E + 1)
                        psr = ps[:].rearrange("p (h c) -> p h c", c=E)
                        bvrr = bvt[:].rearrange("p (h c) -> p h c", c=E)
                        nc.vector.tensor_tensor(vtr[:, :, 0:E], psr, bvrr, op=add)
                        nc.vector.memset(vtr[:, :, E], 1.0)
                units.append(lambda: go(0))
                units.append(lambda: go(1))
                return units

            def dpv_unit(pc, pi):
                def go():
                    pa, pb = PASSES[pi]
                    sa, sb = "hl"[pa], "hl"[pb]
                    for tp in range(6):
                        lhsT = p8[sa][tp][:].rearrange(
                            "p (j m) -> p j m", m=512)[:, :, P * pc:P * (pc + 1)]
                        rhs = dt8w[sb][:, 1024 * tp:1024 * (tp + 1)].rearrange(
                            "p (j q) -> p j q", q=512)
                        nc.tensor.matmul(
                            a2w[pc][:], lhsT, rhs,
                            start=(pi == 0 and tp == 0), stop=False,
                            perf_mode=DR)
                return go

            # ---- prologue: Q/K projections for q-block 0 (all Q first so
            # the K weight/x DMAs hide behind Q compute) --------------------
            for p in range(4):
                for u_ in proj_units(p, 0, "q", proto=True):
                    u_()
            for p in range(4):
                for u_ in proj_units(p, 0, "k", proto=True):
                    u_()

            # ---- attention: flat pipeline over steps s = (i, p) ------------
            # scores of step s interleave with PV of step s-1 (exp stream on
            # the scalar engine never pauses) plus proj/D@PV fillers.
            def pv_group(ip, pp, par, jj):
                def go():
                    j = 4 * ip + jj
                    s0 = 130 * (pvslot[0] % 3)
                    pvslot[0] += 1
                    pvs = pvt[:, s0:s0 + 130]
                    for h in range(2):
                        for c in range(j + 1):
                            nc.tensor.matmul(
                                pvs[:, 65 * h:65 * (h + 1)],
                                uchunk(par, c, h, jj),
                                vt[c][:, 65 * (2 * pp + h):65 * (2 * pp + h + 1)],
                                start=(c == 0), stop=(c == j))
                    pvr = pvs.rearrange("p (h c) -> p h c", c=E + 1)
                    rcp = rpool.tile([P, 2], f32, tag="rcp", name="rcp")
                    nc.vector.reciprocal(rcp[:], pvr[:, :, E])
                    for h in range(2):
                        nc.vector.tensor_scalar(
                            pvg[j][:, E * (2 * pp + h):E * (2 * pp + h + 1)],
                            pvs[:, 65 * h:65 * h + E],
                            rcp[:, h:h + 1], None, op0=mult)
                    if pp == 3:  # pvg[j] complete: fp8 hi/lo for the DR
                        # tail. GpSimd (SBUF-only, mostly idle); the last
                        # four gate the tail start, so those go on DVE
                        eng = nc.vector if j >= 12 else nc.gpsimd
                        s8 = slice(512 * (j % 2), 512 * (j % 2) + 512)
                        eng.tensor_copy(p8["h"][j // 2][:, s8], pvg[j][:])
                        eng.tensor_tensor(
                            p8["l"][j // 2][:, s8], pvg[j][:],
                            p8["h"][j // 2][:, s8], op=mybir.AluOpType.subtract)
                return go

            step = 0
            pvq = []
            for i in range(4):
                if i < 3:
                    issue_x(i + 1)
                fillers = []
                for tcc in range(4):
                    fillers += v_proj_units(i, tcc)
                if i == 3:
                    issue_dt8w()
                    for pc in range(2):
                        for pi in range(3):
                            fillers.append(dpv_unit(pc, pi))
                if i < 3:
                    for p in range(4):
                        fillers += proj_units(p, i + 1, "q")
                # K projections of THIS block are emitted lazily inside each
                # step, right before its first diagonal score chunk
                kunits = {p: (proj_units(p, i, "k") if i >= 1 else [])
                          for p in range(4)}
                fp = [0]

                def pump(k, fillers=fillers, fp=fp):
                    while k > 0 and fp[0] < len(fillers):
                        fillers[fp[0]]()
                        fp[0] += 1
                        k -= 1

                def score_chunk(ii, pp, par, c):
                    j0 = max(0, c - 4 * ii)
                    sp = sps.tile([P, 1024], f32, tag="sp", name="sp")
                    for h in range(2):
                        nc.tensor.matmul(
                            sp[:, 512 * h + P * j0:512 * (h + 1)],
                            kt[pp][64 * h:64 * (h + 1), P * c:P * (c + 1)],
                            qt[pp][64 * h:64 * (h + 1),
                                   512 * ii + P * j0:512 * (ii + 1)],
                            start=True, stop=True)
                    spr = sp[:].rearrange("p (h w) -> p h w", w=512)
                    ur = ut2[par][c][:].rearrange("p (h w) -> p h w", w=512)
                    nc.scalar.activation(
                        ur[:, :, P * j0:512], spr[:, :, P * j0:512],
                        Exp, scale=COEF / (SCL * SCL))
                    if c >= 4 * ii:
                        # mask on GpSimd: SBUF-only, engine otherwise idle
                        nc.gpsimd.tensor_tensor(
                            ur[:, :, P * j0:P * (j0 + 1)],
                            ur[:, :, P * j0:P * (j0 + 1)], mskr, op=mult)

                nch = 4 * (i + 1)
                for p in range(4):
                    par = step % 2
                    c0 = PRE3 if (i == 3 and p == 0) else 0
                    for c in range(c0, nch):
                        if c == c0 + 1:  # K proj of this block before diag use
                            for u_ in kunits[p]:
                                u_()
                            kunits[p] = []
                        score_chunk(i, p, par, c)
                        if c % 2 == 0 and pvq:
                            pvq.pop(0)()
                        elif c % 2 == 1:
                            pump(1)
                    while pvq:
                        pvq.pop(0)()
                    if p == 0:
                        pump(8)  # all V-proj of this i before its PV starts
                    pump((len(fillers) * (p + 1)) // 4 - fp[0])
                    pvq = [pv_group(i, p, par, jj) for jj in range(4)]
                    step += 1
                pump(len(fillers))
                if i == 2:
                    # pre-compute the first exp chunks of block 3's first
                    # step in block 2's scalar-engine slack (par of step 12
                    # is 0; its gen-0 u tiles are free by now), interleaved
                    # with the pending PV groups of step (2,3)
                    for c in range(PRE3):
                        score_chunk(3, 0, 0, c)
                        if c % 2 == 0 and pvq:
                            pvq.pop(0)()
            while pvq:
                pvq.pop(0)()

        # ---- tail: finish D@PV + output projection, both fp8 DoubleRow -----
        with tc.tile_pool(name="dtp", bufs=2) as dtp, \
             tc.tile_pool(name="wo", bufs=1) as wop, \
             tc.tile_pool(name="a2s", bufs=1) as a2sp, \
             tc.tile_pool(name="obuf", bufs=3) as obp, \
             tc.tile_pool(name="aps", bufs=1, space="PSUM") as aps, \
             tc.tile_pool(name="ops", bufs=2, space="PSUM") as ops:
            wo8 = {}
            for hl in "hl":
                w = wop.tile([P, 4 * D], f8, tag="wo" + hl, name="wo" + hl)
                nc.sync.dma_start(
                    w[:].rearrange("p (c q) -> p c q", q=D),
                    w8d["wo" + hl][:].rearrange("(c p) q -> p c q", p=P))
                wo8[hl] = w

            dtb8 = {}

            def issue_dtb8(qb):
                pr = {}
                for hl in "hl":
                    b = dtp.tile([P, NTC * 512], f8, tag="dtb" + hl,
                                 name="dtb" + hl)
                    nc.sync.dma_start(
                        b[:].rearrange("p (t q) -> p t q", q=512),
                        w8d["dt" + hl][:, 512 * qb:512 * (qb + 1)].rearrange(
                            "(t p) q -> p t q", p=P))
                    pr[hl] = b
                dtb8[qb] = pr

            a2s8 = {}

            def a2s_copy(qb, pc, src):
                if qb not in a2s8:
                    a2s8[qb] = {hl: a2sp.tile([P, 2048], f8,
                                              tag=f"a2s{hl}{qb}",
                                              name=f"a2s{hl}{qb}")
                                for hl in "hl"}
                dh = a2s8[qb]["h"][:, 512 * pc:512 * (pc + 1)]
                dl = a2s8[qb]["l"][:, 512 * pc:512 * (pc + 1)]
                nc.scalar.activation(dh, src[:], Copy)
                nc.vector.tensor_tensor(
                    dl, src[:], dh, op=mybir.AluOpType.subtract)

            def dpv_tail(qb, pcs):
                a2 = {pc: aps.tile([P, 512], f32, tag=f"a2t{pc}", name="a2t")
                      for pc in pcs}
                for pc in pcs:
                    n = 0
                    for pa, pb in PASSES:
                        sa, sb = "hl"[pa], "hl"[pb]
                        for tp in range(8):
                            lhsT = p8[sa][tp][:].rearrange(
                                "p (j m) -> p j m", m=512)[:, :, P * pc:P * (pc + 1)]
                            rhs = dtb8[qb][sb][:, 1024 * tp:1024 * (tp + 1)].rearrange(
                                "p (j q) -> p j q", q=512)
                            nc.tensor.matmul(
                                a2[pc][:], lhsT, rhs,
                                start=(n == 0), stop=(n == 23), perf_mode=DR)
                            n += 1
                    a2s_copy(qb, pc, a2[pc])

            def out_proj(qb, qss=range(4)):
                for qs in qss:
                    ob = obp.tile([P, D], bf16, tag="ob", name="ob")
                    for nh in range(2):
                        op_ = ops.tile([P, 512], f32, tag="op", name="op")
                        n = 0
                        for pa, pb in PASSES:
                            sa, sb = "hl"[pa], "hl"[pb]
                            for kk in range(2):
                                lhsT = a2s8[qb][sa][:, 1024 * kk:1024 * (kk + 1)
                                                    ].rearrange(
                                    "p (j m) -> p j m", m=512)[
                                    :, :, P * qs:P * (qs + 1)]
                                rhs = wo8[sb][:, 2048 * kk:2048 * (kk + 1)
                                              ].rearrange(
                                    "p (j q) -> p j q", q=D)[
                                    :, :, 512 * nh:512 * (nh + 1)]
                                nc.tensor.matmul(
                                    op_[:], lhsT, rhs,
                                    start=(n == 0), stop=(n == 5), perf_mode=DR)
                                n += 1
                        if qb < 2:
                            nc.scalar.activation(
                                ob[:, 512 * nh:512 * (nh + 1)], op_[:], Copy,
                                scale=1.0 / 8192.0)
                        else:
                            nc.vector.tensor_scalar(
                                ob[:, 512 * nh:512 * (nh + 1)], op_[:],
                                1.0 / 8192.0, None, op0=mult)
                        nc.sync.dma_start(
                            y[512 * qb + P * qs:512 * qb + P * (qs + 1),
                              512 * nh:512 * (nh + 1)],
                            ob[:, 512 * nh:512 * (nh + 1)])

            issue_dtb8(0)
            # finish the in-window accumulators (qb0, pc 0/1): t = 12..15
            for pc in range(2):
                n = 0
                for pa, pb in PASSES:
                    sa, sb = "hl"[pa], "hl"[pb]
                    for tp in (6, 7):
                        lhsT = p8[sa][tp][:].rearrange(
                            "p (j m) -> p j m", m=512)[:, :, P * pc:P * (pc + 1)]
                        rhs = dtb8[0][sb][:, 1024 * tp:1024 * (tp + 1)
                                          ].rearrange("p (j q) -> p j q", q=512)
                        nc.tensor.matmul(
                            a2w[pc][:], lhsT, rhs,
                            start=False, stop=(n == 5), perf_mode=DR)
                        n += 1
            for pc in range(2):
                a2s_copy(0, pc, a2w[pc])
            dpv_tail(0, [2, 3])
            issue_dtb8(1)
            dpv_tail(1, [0, 1, 2, 3])
            out_proj(0)
            issue_dtb8(2)
            dpv_tail(2, [0, 1, 2, 3])
            out_proj(1)
            issue_dtb8(3)
            dpv_tail(3, [0, 1])
            out_proj(2, (0, 1))
            dpv_tail(3, [2, 3])
            out_proj(2, (2, 3))
            out_proj(3)

    nc.compile()
    return nc


F8NP = ml_dtypes.float8_e4m3


def _f8split(a):
    """(hi, lo) fp8 e4m3 split: hi + lo ~ a with ~0.1% relative error."""
    hi = np.asarray(a, F8NP)
    lo = np.asarray(a - hi.astype(np.float32), F8NP)
    return hi, lo


def _prep_inputs(query_1, key_1, value_1, Wq, bq, Wk, bk, Wv, bv, Wo, bo, Dmat):
    """Host-side sharding + fp8/bf16 packing: per-core input dicts."""
    f = np.float32

    def xT(x, b):
        return _f8split(np.ascontiguousarray(np.asarray(x[b], f).T))

    per_g = []
    for g in range(2):
        h0 = HG * g
        wqg = np.zeros((D, 512), f)
        wkg = np.zeros((D, 512), f)
        bqg = np.zeros((P, 4), f)
        bkg = np.zeros((P, 4), f)
        for p in range(4):
            for h in range(2):
                hh = h0 + 2 * p + h
                c0 = 128 * p + 64 * h
                wqg[:, c0:c0 + 64] = np.asarray(Wq[hh], f).T * SCL
                wkg[:, c0:c0 + 64] = np.asarray(Wk[hh], f).T * SCL
                bqg[64 * h:64 * (h + 1), p] = np.asarray(bq[hh], f) * SCL
                bkg[64 * h:64 * (h + 1), p] = np.asarray(bk[hh], f) * SCL
        wvg = np.zeros((D, 512), f)
        bvg = np.zeros((512,), f)
        for j in range(HG):
            wvg[:, 64 * j:64 * (j + 1)] = np.asarray(Wv[h0 + j], f).T * SCL
            bvg[64 * j:64 * (j + 1)] = np.asarray(bv[h0 + j], f) * SCL
        wog = np.ascontiguousarray(
            np.asarray(Wo, f)[:, 64 * h0:64 * h0 + 512].T) * SCL
        gd = {}
        gd["wqh"], gd["wql"] = _f8split(wqg)
        gd["wkh"], gd["wkl"] = _f8split(wkg)
        gd["wvh"], gd["wvl"] = _f8split(wvg)
        gd["woh"], gd["wol"] = _f8split(wog)
        gd["bqc"], gd["bkc"] = bqg, bkg
        gd["bvr"] = np.broadcast_to(bvg, (P, 512)).astype(BF)
        per_g.append(gd)

    dTs = np.ascontiguousarray(np.asarray(Dmat, f).T) * SCLD
    dth, dtl_ = _f8split(dTs)
    m = (np.arange(128)[None, :] >= np.arange(128)[:, None]).astype(f)  # [k, q]
    msk2 = np.ascontiguousarray(np.tile(m, (1, 2))).astype(BF)

    xs = {}
    for nm, src in (("xq", query_1), ("xk", key_1), ("xv", value_1)):
        for b in range(B):
            xs[(nm, b, "h")], xs[(nm, b, "l")] = xT(src, b)

    in_maps = []
    for c in range(8):
        b, g = c // 2, c % 2
        im = {"dth": dth, "dtl": dtl_, "msk": msk2}
        im.update(per_g[g])
        for nm in ("xq", "xk", "xv"):
            im[nm + "h"] = xs[(nm, b, "h")]
            im[nm + "l"] = xs[(nm, b, "l")]
        in_maps.append(im)
    return in_maps


def kernel(query_1, key_1, value_1, Wq, bq, Wk, bk, Wv, bv, Wo, bo, D):
    import os
    os.environ["BASS_NEVER_TRACE"] = "1"  # NTFF capture hangs over the axon relay
    global _CACHED_NC
    if _CACHED_NC is None:
        _CACHED_NC = _build_nc()
    nc = _CACHED_NC
    in_maps = _prep_inputs(query_1, key_1, value_1, Wq, bq, Wk, bk, Wv, bv, Wo, bo, D)
    res = run_bass_kernel_spmd(nc, in_maps, core_ids=list(range(8)))
    bo_f = np.asarray(bo, np.float32)
    out = np.empty((B, T, 1024), np.float32)
    for b in range(B):
        out[b] = (res.results[2 * b]["y"].astype(np.float32)
                  + res.results[2 * b + 1]["y"].astype(np.float32) + bo_f)
    return out


# revision 91
# speedup vs baseline: 1.0001x; 1.0001x over previous
"""Trainium2 Bass kernel: causal multi-head attention with an extra time-mixing
matrix D (attn = D @ softmax(mask(Q K^T / sqrt(e))) @ V, concat heads, out proj).

Shapes (hardcoded): B=4, T=2048, d=1024, H=16, e=64, fp32 in/out.
Sharding over 8 NeuronCores: data-parallel over batch (4) x tensor-parallel over
heads (2 groups of 8). Each core computes its batch/head-group partial
y_part = concat(attn_heads) @ Wo_part^T; host sums the 2 partials per batch and
adds bo.

All matmul data is bf16 (host-converted). Matmul cost on PE is out-free-size
cycles regardless of contraction depth, so PV runs with 65-wide moving ([V|1]
per head; the ones column accumulates softmax row sums in the same matmul)
against stationary exp-score chunks, and scores skip fully-masked 128x128
blocks. exp runs on the scalar engine straight out of PSUM with causal-ragged
access patterns; only diagonal 128x128 blocks need the mask multiply.

Schedule: i-outer (query blocks), p-inner (head pairs). The exp stream keeps
the scalar engine near-saturated during attention, so all remaining PE work is
fed in as fillers between score chunks: V projection for this block, Q/K
projections for the next block, and D@PV accumulation for half the output
columns (a2 psum banks held across the whole loop). The tail finishes D@PV for
the other half plus the output projection. All DMAs are batched multi-dim
transfers (HWDGE charges a fixed ~625ns per DMA instruction).
"""

import sys

for _p in ("/opt/trn_rl_repo", "/root/.axon_site/_ro/trn_rl_repo"):
    if _p not in sys.path:
        sys.path.append(_p)

from contextlib import ExitStack

import ml_dtypes
import numpy as np

import concourse.bass as bass  # noqa: F401  (AP helpers)
import concourse.tile as tile
from concourse import bacc, mybir
from concourse.bass_utils import run_bass_kernel_spmd

dt = mybir.dt
BF = ml_dtypes.bfloat16

B, T, D, H, E = 4, 2048, 1024, 16, 64
HG = 8          # heads per core (tensor-parallel group)
COEF = 1.0 / E ** 0.5
P = 128         # partitions
SCL = 32.0      # Wq/Wk/Wv/Wo host-side scale: keeps fp8 hi/lo splits in
                # normal range; exp scale and the final Copy compensate
SCLD = 8.0      # D host-side scale (a2 psum stays under fp8 max 448)
NTC = T // P    # 16 time chunks
PRE3 = 8        # exp chunks of step (3,0) precomputed in block 2's slack
ND = D // P     # 8 contraction chunks (d)

_CACHED_NC = None


def _build_nc():
    """Build + compile the single-core program (same NEFF on all 8 cores)."""
    nc = bacc.Bacc("TRN2", target_bir_lowering=False, debug=False)
    f32, bf16 = dt.float32, dt.bfloat16
    Exp = mybir.ActivationFunctionType.Exp
    Copy = mybir.ActivationFunctionType.Copy
    mult = mybir.AluOpType.mult
    add = mybir.AluOpType.add

    f8 = dt.float8e4
    DR = mybir.MatmulPerfMode.DoubleRow
    x8d, w8d = {}, {}
    for nm in ("xq", "xk", "xv"):
        for hl in "hl":
            x8d[nm + hl] = nc.dram_tensor(
                nm + hl, [D, T], f8, kind="ExternalInput").ap()
    for nm in ("wq", "wk", "wv"):
        for hl in "hl":
            w8d[nm + hl] = nc.dram_tensor(
                nm + hl, [D, 512], f8, kind="ExternalInput").ap()
    for nm in ("wo", "dt"):
        for hl in "hl":
            w8d[nm + hl] = nc.dram_tensor(
                nm + hl, [512, D] if nm == "wo" else [T, T], f8,
                kind="ExternalInput").ap()
    msk = nc.dram_tensor("msk", [P, 256], bf16, kind="ExternalInput").ap()
    bqc = nc.dram_tensor("bqc", [P, 4], f32, kind="ExternalInput").ap()
    bkc = nc.dram_tensor("bkc", [P, 4], f32, kind="ExternalInput").ap()
    bvr = nc.dram_tensor("bvr", [P, 512], bf16, kind="ExternalInput").ap()
    y = nc.dram_tensor("y", [T, D], bf16, kind="ExternalOutput").ap()

    with tile.TileContext(nc) as tc, ExitStack() as ctx:
        # ---- constants + persistent tiles ----------------------------------
        consts = ctx.enter_context(tc.tile_pool(name="consts", bufs=1))
        mskt = consts.tile([P, 256], bf16, tag="mskt")
        bqt = consts.tile([P, 4], f32, tag="bqt")
        bkt = consts.tile([P, 4], f32, tag="bkt")
        bvt = consts.tile([P, 512], bf16, tag="bvt")
        warm = consts.tile([P, 1], f32, tag="warm")

        persist = ctx.enter_context(tc.tile_pool(name="persist", bufs=1))
        qt = [persist.tile([P, T], bf16, tag=f"qt{p}", name=f"qt{p}") for p in range(4)]
        kt = [persist.tile([P, T], bf16, tag=f"kt{p}", name=f"kt{p}") for p in range(4)]
        # vt: per t-chunk [V_h | 1] blocks: col 65*h+e = V, col 65*h+64 = 1.0
        vt = [persist.tile([P, HG * (E + 1)], bf16, tag=f"vt{t}", name=f"vt{t}")
              for t in range(NTC)]
        # pvg: normalized softmax(S) @ V staging (heads side by side); only
        # feeds the fp8 hi/lo requantization now, so a ring of 8 suffices
        pvgr = [persist.tile([P, 512], bf16, tag=f"pvg{t}", name=f"pvg{t}")
                for t in range(8)]
        pvg = {j: pvgr[j % 8] for j in range(NTC)}
        # fp8 hi/lo requantization of pvg, packed in t-chunk PAIRS for the
        # DoubleRow D@PV in the tail; filled by DVE as pvg completes
        p8 = {hl: [persist.tile([P, 1024], dt.float8e4, tag=f"p8{hl}{tp}",
                                name=f"p8{hl}{tp}")
                   for tp in range(NTC // 2)] for hl in "hl"}

        # a2 accumulators for (qb=0, pc=0/1) — held across the attention loop
        # so part of D@PV streams in as attention fillers (2 psum banks).
        a2ps = ctx.enter_context(tc.tile_pool(name="a2ps", bufs=1, space="PSUM"))
        a2w = [a2ps.tile([P, 512], f32, tag=f"a2w{q}", name=f"a2w{q}")
               for q in range(2)]

        with tc.tile_pool(name="wpool", bufs=1) as wpool, \
             tc.tile_pool(name="xpool", bufs=1) as xpool, \
             tc.tile_pool(name="dtw", bufs=1) as dtwp, \
             tc.tile_pool(name="upool", bufs=1) as upool, \
             tc.tile_pool(name="rpool", bufs=4) as rpool, \
             tc.tile_pool(name="sps", bufs=2, space="PSUM") as sps, \
             tc.tile_pool(name="pvps", bufs=1, space="PSUM") as pvps, \
             tc.tile_pool(name="pjps", bufs=1, space="PSUM") as pjps:

            def load_wb(nm, split=False):
                tiles = []
                for hl in "hl":
                    wdram = w8d[nm + hl]
                    wb = wpool.tile([P, ND * 512], f8, tag=nm + hl, name=nm + hl)
                    nd = 2 if split else 1
                    hd = ND // nd
                    for s in range(nd):
                        nc.sync.dma_start(
                            wb[:, 512 * hd * s:512 * hd * (s + 1)].rearrange(
                                "p (d q) -> p d q", q=512),
                            wdram[P * hd * s:P * hd * (s + 1), :].rearrange(
                                "(d p) q -> p d q", p=P))
                    tiles.append(wb)
                    split = False
                return tiles  # [hi tile, lo tile]

            def drslice(tile_, kk, m0=None):
                """[128, 2 k-tiles, m] DoubleRow operand slice for super-chunk kk."""
                a = tile_[:, 1024 * kk:1024 * (kk + 1)].rearrange(
                    "p (j m) -> p j m", m=512)
                return a if m0 is None else a[:, :, m0:m0 + P]

            # wq + xq(0) load first: the prologue's first matmul needs them
            wq = load_wb("wq", split=True)

            # proj psum ring: one pjps bank, plus the two a2w banks which
            # are free until the D@PV fillers start accumulating at i=3
            pj0 = pjps.tile([P, 512], f32, tag="pj", name="pj")
            pjring = [[pj0, a2w[0], a2w[1]], [pj0]]
            pjslot = [0]

            # two generations of exp-score tiles (step parity) so PV of step
            # s-1 can interleave with scores of step s
            ut2 = [[upool.tile([P, 1024], bf16, tag=f"u{g}_{c}", name=f"u{g}_{c}")
                    for c in range(NTC)] for g in range(2)]
            mskr = mskt[:].rearrange("p (s c) -> p s c", c=P)
            pvt = pvps.tile([P, 390], f32, tag="pvt", name="pvt")  # 3 slots x 130
            pvslot = [0]

            def uchunk(par, c, h, jj):
                return ut2[par][c][:, 512 * h + P * jj:512 * h + P * (jj + 1)]

            xcur = {}

            def issue_x1(kind, tb, split=False):
                pair = []
                for hl in "hl":
                    dram = x8d[kind + hl]
                    xb = xpool.tile([P, ND * 512], f8, tag=kind + hl,
                                    name=kind + hl,
                                    bufs=2 if kind == "xk" else 1)
                    nd = 2 if split else 1
                    hd = ND // nd
                    src = dram[:, 512 * tb:512 * (tb + 1)]
                    for s in range(nd):
                        nc.sync.dma_start(
                            xb[:, 512 * hd * s:512 * hd * (s + 1)].rearrange(
                                "p (d q) -> p d q", q=512),
                            src[P * hd * s:P * hd * (s + 1), :].rearrange(
                                "(d p) q -> p d q", p=P))
                    pair.append(xb)
                    split = False
                xcur[(kind, tb)] = pair

            def issue_x(tb):
                for kind in ("xq", "xk", "xv"):
                    issue_x1(kind, tb)

            issue_x1("xq", 0, split=True)
            nc.sync.dma_start(bqt[:], bqc[:])
            wk = load_wb("wk")
            issue_x1("xk", 0)
            nc.sync.dma_start(bkt[:], bkc[:])
            wv = load_wb("wv")
            issue_x1("xv", 0)
            nc.sync.dma_start(bvt[:], bvr[:])
            nc.sync.dma_start(mskt[:], msk[:])
            nc.vector.memset(warm[:], 0.0)
            nc.scalar.activation(warm[:], warm[:], Exp)  # preload exp table

            dt8w = {}

            def issue_dt8w():
                for hl in "hl":
                    b = dtwp.tile([P, 12 * 512], f8, tag="dt8" + hl,
                                  name="dt8w")
                    nc.sync.dma_start(
                        b[:].rearrange("p (t q) -> p t q", q=512),
                        w8d["dt" + hl][0:1536, 0:512].rearrange(
                            "(t p) q -> p t q", p=P))
                    dt8w[hl] = b

            # ---- filler units (each emits ~0.8-2us of PE work) -------------
            # Projections run as fp8 DoubleRow 3-pass (hi*hi + hi*lo + lo*hi):
            # 12 matmuls of 256 k-depth each, 0.5 cycles/row.
            PASSES = ((0, 0), (0, 1), (1, 0))  # (stationary hi/lo, moving hi/lo)

            def proj_units(p, tb, which, proto=False):
                """Q or K projection of q-block tb for head pair p."""
                wt, bt, dest, kind = {
                    "q": (wq, bqt, qt, "xq"), "k": (wk, bkt, kt, "xk")}[which]

                def go(half, holder=[None]):
                    xp = xcur[(kind, tb)]
                    if half == 0:
                        if proto:  # borrow the idle scores ring (4 banks)
                            sp_ = sps.tile([P, 1024], f32, tag="sp", name="sp")
                            holder[0] = sp_[:, 0:512]
                        else:
                            # runs at block tb-1 (Q) / tb (K); a2w banks are
                            # free until the D@PV fillers start at block 3
                            blk = tb - 1 if which == "q" else tb
                            ring = pjring[0 if blk < 3 else 1]
                            holder[0] = ring[pjslot[0] % len(ring)]
                            pjslot[0] += 1
                    ps = holder[0]
                    for idx in range(6 * half, 6 * half + 6):
                        pw, px = PASSES[idx // 4]
                        kk = idx % 4
                        nc.tensor.matmul(
                            ps[:], drslice(wt[pw], kk, P * p),
                            drslice(xp[px], kk),
                            start=(idx == 0), stop=(idx == 11), perf_mode=DR)
                    if half == 1:
                        nc.vector.tensor_scalar(
                            dest[p][:, 512 * tb:512 * (tb + 1)], ps[:],
                            bt[:, p:p + 1], None, op0=add)
                return [lambda: go(0), lambda: go(1)]

            def v_proj_units(tb, tcc):
                """V projection for t-chunk 4*tb+tcc."""
                units = []

                def go(half, holder=[None]):
                    xp = xcur[("xv", tb)]
                    if half == 0:
                        ring = pjring[0 if tb < 3 else 1]
                        holder[0] = ring[pjslot[0] % len(ring)]
                        pjslot[0] += 1
                    ps = holder[0]
                    for idx in range(6 * half, 6 * half + 6):
                        px, pw = PASSES[idx // 4]
                        kk = idx % 4
                        nc.tensor.matmul(
                            ps[:], drslice(xp[px], kk, P * tcc),
                            drslice(wv[pw], kk),
                            start=(idx == 0), stop=(idx == 11), perf_mode=DR)
                    if half == 1:
                        t = 4 * tb + tcc
                        vtr = vt[t][:].rearrange("p (h c) -> p h c", c=E + 1)
                        psr = ps[:].rearrange("p (h c) -> p h c", c=E)
                        bvrr = bvt[:].rearrange("p (h c) -> p h c", c=E)
                        nc.vector.tensor_tensor(vtr[:, :, 0:E], psr, bvrr, op=add)
                        nc.vector.memset(vtr[:, :, E], 1.0)
                units.append(lambda: go(0))
                units.append(lambda: go(1))
                return units

            def dpv_unit(pc, pi):
                def go():
                    pa, pb = PASSES[pi]
                    sa, sb = "hl"[pa], "hl"[pb]
                    for tp in range(6):
                        lhsT = p8[sa][tp][:].rearrange(
                            "p (j m) -> p j m", m=512)[:, :, P * pc:P * (pc + 1)]
                        rhs = dt8w[sb][:, 1024 * tp:1024 * (tp + 1)].rearrange(
                            "p (j q) -> p j q", q=512)
                        nc.tensor.matmul(
                            a2w[pc][:], lhsT, rhs,
                            start=(pi == 0 and tp == 0), stop=False,
                            perf_mode=DR)
                return go

            # ---- prologue: Q/K projections for q-block 0 (all Q first so
            # the K weight/x DMAs hide behind Q compute) --------------------
            for p in range(4):
                for u_ in proj_units(p, 0, "q", proto=True):
                    u_()
            for p in range(4):
                for u_ in proj_units(p, 0, "k", proto=True):
                    u_()

            # ---- attention: flat pipeline over steps s = (i, p) ------------
            # scores of step s interleave with PV of step s-1 (exp stream on
            # the scalar engine never pauses) plus proj/D@PV fillers.
            def pv_group(ip, pp, par, jj):
                def go():
                    j = 4 * ip + jj
                    s0 = 130 * (pvslot[0] % 3)
                    pvslot[0] += 1
                    pvs = pvt[:, s0:s0 + 130]
                    for h in range(2):
                        for c in range(j + 1):
                            nc.tensor.matmul(
                                pvs[:, 65 * h:65 * (h + 1)],
                                uchunk(par, c, h, jj),
                                vt[c][:, 65 * (2 * pp + h):65 * (2 * pp + h + 1)],
                                start=(c == 0), stop=(c == j))
                    pvr = pvs.rearrange("p (h c) -> p h c", c=E + 1)
                    rcp = rpool.tile([P, 2], f32, tag="rcp", name="rcp")
                    nc.vector.reciprocal(rcp[:], pvr[:, :, E])
                    for h in range(2):
                        nc.vector.tensor_scalar(
                            pvg[j][:, E * (2 * pp + h):E * (2 * pp + h + 1)],
                            pvs[:, 65 * h:65 * h + E],
                            rcp[:, h:h + 1], None, op0=mult)
                    if pp == 3:  # pvg[j] complete: fp8 hi/lo for the DR
                        # tail. GpSimd (SBUF-only, mostly idle); the last
                        # four gate the tail start, so those go on DVE
                        eng = nc.vector if j >= 12 else nc.gpsimd
                        s8 = slice(512 * (j % 2), 512 * (j % 2) + 512)
                        eng.tensor_copy(p8["h"][j // 2][:, s8], pvg[j][:])
                        eng.tensor_tensor(
                            p8["l"][j // 2][:, s8], pvg[j][:],
                            p8["h"][j // 2][:, s8], op=mybir.AluOpType.subtract)
                return go

            step = 0
            pvq = []
            for i in range(4):
                if i < 3:
                    issue_x(i + 1)
                fillers = []
                for tcc in range(4):
                    fillers += v_proj_units(i, tcc)
                if i == 3:
                    issue_dt8w()
                    for pc in range(2):
                        for pi in range(3):
                            fillers.append(dpv_unit(pc, pi))
                if i < 3:
                    for p in range(4):
                        fillers += proj_units(p, i + 1, "q")
                # K projections of THIS block are emitted lazily inside each
                # step, right before its first diagonal score chunk
                kunits = {p: (proj_units(p, i, "k") if i >= 1 else [])
                          for p in range(4)}
                fp = [0]

                def pump(k, fillers=fillers, fp=fp):
                    while k > 0 and fp[0] < len(fillers):
                        fillers[fp[0]]()
                        fp[0] += 1
                        k -= 1

                def score_chunk(ii, pp, par, c):
                    j0 = max(0, c - 4 * ii)
                    sp = sps.tile([P, 1024], f32, tag="sp", name="sp")
                    for h in range(2):
                        nc.tensor.matmul(
                            sp[:, 512 * h + P * j0:512 * (h + 1)],
                            kt[pp][64 * h:64 * (h + 1), P * c:P * (c + 1)],
                            qt[pp][64 * h:64 * (h + 1),
                                   512 * ii + P * j0:512 * (ii + 1)],
                            start=True, stop=True)
                    spr = sp[:].rearrange("p (h w) -> p h w", w=512)
                    ur = ut2[par][c][:].rearrange("p (h w) -> p h w", w=512)
                    nc.scalar.activation(
                        ur[:, :, P * j0:512], spr[:, :, P * j0:512],
                        Exp, scale=COEF / (SCL * SCL))
                    if c >= 4 * ii:
                        # mask on GpSimd: SBUF-only, engine otherwise idle
                        nc.gpsimd.tensor_tensor(
                            ur[:, :, P * j0:P * (j0 + 1)],
                            ur[:, :, P * j0:P * (j0 + 1)], mskr, op=mult)

                nch = 4 * (i + 1)
                for p in range(4):
                    par = step % 2
                    c0 = PRE3 if (i == 3 and p == 0) else 0
                    for c in range(c0, nch):
                        if c == c0 + 1:  # K proj of this block before diag use
                            for u_ in kunits[p]:
                                u_()
                            kunits[p] = []
                        score_chunk(i, p, par, c)
                        if c % 2 == 0 and pvq:
                            pvq.pop(0)()
                        elif c % 2 == 1:
                            pump(1)
                    while pvq:
                        pvq.pop(0)()
                    if p == 0:
                        pump(8)  # all V-proj of this i before its PV starts
                    pump((len(fillers) * (p + 1)) // 4 - fp[0])
                    pvq = [pv_group(i, p, par, jj) for jj in range(4)]
                    step += 1
                if i == 2:
                    # pre-compute the first exp chunks of block 3's first
                    # step in block 2's scalar-engine slack (par of step 12
                    # is 0; its gen-0 u tiles are free by now), interleaved
                    # with the pending PV groups of step (2,3) and leftover
                    # fillers so the exp pipeline stalls stay absorbed
                    for c in range(PRE3):
                        score_chunk(3, 0, 0, c)
                        if c % 2 == 0 and pvq:
                            pvq.pop(0)()
                        elif c % 2 == 1:
                            pump(1)
                pump(len(fillers))
            while pvq:
                pvq.pop(0)()

        # ---- tail: finish D@PV + output projection, both fp8 DoubleRow -----
        with tc.tile_pool(name="dtp", bufs=3) as dtp, \
             tc.tile_pool(name="wo", bufs=1) as wop, \
             tc.tile_pool(name="a2s", bufs=1) as a2sp, \
             tc.tile_pool(name="obuf", bufs=3) as obp, \
             tc.tile_pool(name="aps", bufs=1, space="PSUM") as aps, \
             tc.tile_pool(name="ops", bufs=2, space="PSUM") as ops:
            wo8 = {}
            for hl in "hl":
                w = wop.tile([P, 4 * D], f8, tag="wo" + hl, name="wo" + hl)
                nc.sync.dma_start(
                    w[:].rearrange("p (c q) -> p c q", q=D),
                    w8d["wo" + hl][:].rearrange("(c p) q -> p c q", p=P))
                wo8[hl] = w

            dtb8 = {}

            def issue_dtb8(qb):
                pr = {}
                for hl in "hl":
                    b = dtp.tile([P, NTC * 512], f8, tag="dtb" + hl,
                                 name="dtb" + hl)
                    nc.sync.dma_start(
                        b[:].rearrange("p (t q) -> p t q", q=512),
                        w8d["dt" + hl][:, 512 * qb:512 * (qb + 1)].rearrange(
                            "(t p) q -> p t q", p=P))
                    pr[hl] = b
                dtb8[qb] = pr

            a2s8 = {}

            def a2s_copy(qb, pc, src):
                if qb not in a2s8:
                    a2s8[qb] = {hl: a2sp.tile([P, 2048], f8,
                                              tag=f"a2s{hl}{qb}",
                                              name=f"a2s{hl}{qb}")
                                for hl in "hl"}
                dh = a2s8[qb]["h"][:, 512 * pc:512 * (pc + 1)]
                dl = a2s8[qb]["l"][:, 512 * pc:512 * (pc + 1)]
                nc.scalar.activation(dh, src[:], Copy)
                nc.vector.tensor_tensor(
                    dl, src[:], dh, op=mybir.AluOpType.subtract)

            def dpv_tail(qb, pcs):
                a2 = {pc: aps.tile([P, 512], f32, tag=f"a2t{pc}", name="a2t")
                      for pc in pcs}
                for pc in pcs:
                    n = 0
                    for pa, pb in PASSES:
                        sa, sb = "hl"[pa], "hl"[pb]
                        for tp in range(8):
                            lhsT = p8[sa][tp][:].rearrange(
                                "p (j m) -> p j m", m=512)[:, :, P * pc:P * (pc + 1)]
                            rhs = dtb8[qb][sb][:, 1024 * tp:1024 * (tp + 1)].rearrange(
                                "p (j q) -> p j q", q=512)
                            nc.tensor.matmul(
                                a2[pc][:], lhsT, rhs,
                                start=(n == 0), stop=(n == 23), perf_mode=DR)
                            n += 1
                    a2s_copy(qb, pc, a2[pc])

            def out_proj(qb, qss=range(4)):
                for qs in qss:
                    ob = obp.tile([P, D], bf16, tag="ob", name="ob")
                    for nh in range(2):
                        op_ = ops.tile([P, 512], f32, tag="op", name="op")
                        n = 0
                        for pa, pb in PASSES:
                            sa, sb = "hl"[pa], "hl"[pb]
                            for kk in range(2):
                                lhsT = a2s8[qb][sa][:, 1024 * kk:1024 * (kk + 1)
                                                    ].rearrange(
                                    "p (j m) -> p j m", m=512)[
                                    :, :, P * qs:P * (qs + 1)]
                                rhs = wo8[sb][:, 2048 * kk:2048 * (kk + 1)
                                              ].rearrange(
                                    "p (j q) -> p j q", q=D)[
                                    :, :, 512 * nh:512 * (nh + 1)]
                                nc.tensor.matmul(
                                    op_[:], lhsT, rhs,
                                    start=(n == 0), stop=(n == 5), perf_mode=DR)
                                n += 1
                        if qb < 2:
                            nc.scalar.activation(
                                ob[:, 512 * nh:512 * (nh + 1)], op_[:], Copy,
                                scale=1.0 / 8192.0)
                        else:
                            nc.vector.tensor_scalar(
                                ob[:, 512 * nh:512 * (nh + 1)], op_[:],
                                1.0 / 8192.0, None, op0=mult)
                        nc.sync.dma_start(
                            y[512 * qb + P * qs:512 * qb + P * (qs + 1),
                              512 * nh:512 * (nh + 1)],
                            ob[:, 512 * nh:512 * (nh + 1)])

            issue_dtb8(0)
            # finish the in-window accumulators (qb0, pc 0/1): t = 12..15
            for pc in range(2):
                n = 0
                for pa, pb in PASSES:
                    sa, sb = "hl"[pa], "hl"[pb]
                    for tp in (6, 7):
                        lhsT = p8[sa][tp][:].rearrange(
                            "p (j m) -> p j m", m=512)[:, :, P * pc:P * (pc + 1)]
                        rhs = dtb8[0][sb][:, 1024 * tp:1024 * (tp + 1)
                                          ].rearrange("p (j q) -> p j q", q=512)
                        nc.tensor.matmul(
                            a2w[pc][:], lhsT, rhs,
                            start=False, stop=(n == 5), perf_mode=DR)
                        n += 1
            for pc in range(2):
                a2s_copy(0, pc, a2w[pc])
            dpv_tail(0, [2, 3])
            issue_dtb8(1)
            dpv_tail(1, [0, 1, 2, 3])
            out_proj(0)
            issue_dtb8(2)
            dpv_tail(2, [0, 1, 2, 3])
            out_proj(1)
            issue_dtb8(3)
            dpv_tail(3, [0, 1])
            out_proj(2, (0, 1))
            dpv_tail(3, [2, 3])
            out_proj(2, (2, 3))
            out_proj(3)

    nc.compile()
    return nc


F8NP = ml_dtypes.float8_e4m3


def _f8split(a):
    """(hi, lo) fp8 e4m3 split: hi + lo ~ a with ~0.1% relative error."""
    hi = np.asarray(a, F8NP)
    lo = np.asarray(a - hi.astype(np.float32), F8NP)
    return hi, lo


def _prep_inputs(query_1, key_1, value_1, Wq, bq, Wk, bk, Wv, bv, Wo, bo, Dmat):
    """Host-side sharding + fp8/bf16 packing: per-core input dicts."""
    f = np.float32

    def xT(x, b):
        return _f8split(np.ascontiguousarray(np.asarray(x[b], f).T))

    per_g = []
    for g in range(2):
        h0 = HG * g
        wqg = np.zeros((D, 512), f)
        wkg = np.zeros((D, 512), f)
        bqg = np.zeros((P, 4), f)
        bkg = np.zeros((P, 4), f)
        for p in range(4):
            for h in range(2):
                hh = h0 + 2 * p + h
                c0 = 128 * p + 64 * h
                wqg[:, c0:c0 + 64] = np.asarray(Wq[hh], f).T * SCL
                wkg[:, c0:c0 + 64] = np.asarray(Wk[hh], f).T * SCL
                bqg[64 * h:64 * (h + 1), p] = np.asarray(bq[hh], f) * SCL
                bkg[64 * h:64 * (h + 1), p] = np.asarray(bk[hh], f) * SCL
        wvg = np.zeros((D, 512), f)
        bvg = np.zeros((512,), f)
        for j in range(HG):
            wvg[:, 64 * j:64 * (j + 1)] = np.asarray(Wv[h0 + j], f).T * SCL
            bvg[64 * j:64 * (j + 1)] = np.asarray(bv[h0 + j], f) * SCL
        wog = np.ascontiguousarray(
            np.asarray(Wo, f)[:, 64 * h0:64 * h0 + 512].T) * SCL
        gd = {}
        gd["wqh"], gd["wql"] = _f8split(wqg)
        gd["wkh"], gd["wkl"] = _f8split(wkg)
        gd["wvh"], gd["wvl"] = _f8split(wvg)
        gd["woh"], gd["wol"] = _f8split(wog)
        gd["bqc"], gd["bkc"] = bqg, bkg
        gd["bvr"] = np.broadcast_to(bvg, (P, 512)).astype(BF)
        per_g.append(gd)

    dTs = np.ascontiguousarray(np.asarray(Dmat, f).T) * SCLD
    dth, dtl_ = _f8split(dTs)
    m = (np.arange(128)[None, :] >= np.arange(128)[:, None]).astype(f)  # [k, q]
    msk2 = np.ascontiguousarray(np.tile(m, (1, 2))).astype(BF)

    xs = {}
    for nm, src in (("xq", query_1), ("xk", key_1), ("xv", value_1)):
        for b in range(B):
            xs[(nm, b, "h")], xs[(nm, b, "l")] = xT(src, b)

    in_maps = []
    for c in range(8):
        b, g = c // 2, c % 2
        im = {"dth": dth, "dtl": dtl_, "msk": msk2}
        im.update(per_g[g])
        for nm in ("xq", "xk", "xv"):
            im[nm + "h"] = xs[(nm, b, "h")]
            im[nm + "l"] = xs[(nm, b, "l")]
        in_maps.append(im)
    return in_maps


def kernel(query_1, key_1, value_1, Wq, bq, Wk, bk, Wv, bv, Wo, bo, D):
    import os
    os.environ["BASS_NEVER_TRACE"] = "1"  # NTFF capture hangs over the axon relay
    global _CACHED_NC
    if _CACHED_NC is None:
        _CACHED_NC = _build_nc()
    nc = _CACHED_NC
    in_maps = _prep_inputs(query_1, key_1, value_1, Wq, bq, Wk, bk, Wv, bv, Wo, bo, D)
    res = run_bass_kernel_spmd(nc, in_maps, core_ids=list(range(8)))
    bo_f = np.asarray(bo, np.float32)
    out = np.empty((B, T, 1024), np.float32)
    for b in range(B):
        out[b] = (res.results[2 * b]["y"].astype(np.float32)
                  + res.results[2 * b + 1]["y"].astype(np.float32) + bo_f)
    return out


# revision 92
# speedup vs baseline: 1.0019x; 1.0018x over previous
"""Trainium2 Bass kernel: causal multi-head attention with an extra time-mixing
matrix D (attn = D @ softmax(mask(Q K^T / sqrt(e))) @ V, concat heads, out proj).

Shapes (hardcoded): B=4, T=2048, d=1024, H=16, e=64, fp32 in/out.
Sharding over 8 NeuronCores: data-parallel over batch (4) x tensor-parallel over
heads (2 groups of 8). Each core computes its batch/head-group partial
y_part = concat(attn_heads) @ Wo_part^T; host sums the 2 partials per batch and
adds bo.

All matmul data is bf16 (host-converted). Matmul cost on PE is out-free-size
cycles regardless of contraction depth, so PV runs with 65-wide moving ([V|1]
per head; the ones column accumulates softmax row sums in the same matmul)
against stationary exp-score chunks, and scores skip fully-masked 128x128
blocks. exp runs on the scalar engine straight out of PSUM with causal-ragged
access patterns; only diagonal 128x128 blocks need the mask multiply.

Schedule: i-outer (query blocks), p-inner (head pairs). The exp stream keeps
the scalar engine near-saturated during attention, so all remaining PE work is
fed in as fillers between score chunks: V projection for this block, Q/K
projections for the next block, and D@PV accumulation for half the output
columns (a2 psum banks held across the whole loop). The tail finishes D@PV for
the other half plus the output projection. All DMAs are batched multi-dim
transfers (HWDGE charges a fixed ~625ns per DMA instruction).
"""

import sys

for _p in ("/opt/trn_rl_repo", "/root/.axon_site/_ro/trn_rl_repo"):
    if _p not in sys.path:
        sys.path.append(_p)

from contextlib import ExitStack

import ml_dtypes
import numpy as np

import concourse.bass as bass  # noqa: F401  (AP helpers)
import concourse.tile as tile
from concourse import bacc, mybir
from concourse.bass_utils import run_bass_kernel_spmd

dt = mybir.dt
BF = ml_dtypes.bfloat16

B, T, D, H, E = 4, 2048, 1024, 16, 64
HG = 8          # heads per core (tensor-parallel group)
COEF = 1.0 / E ** 0.5
P = 128         # partitions
SCL = 32.0      # Wq/Wk/Wv/Wo host-side scale: keeps fp8 hi/lo splits in
                # normal range; exp scale and the final Copy compensate
SCLD = 8.0      # D host-side scale (a2 psum stays under fp8 max 448)
NTC = T // P    # 16 time chunks
PRE3 = 8        # exp chunks of step (3,0) precomputed in block 2's slack
ND = D // P     # 8 contraction chunks (d)

_CACHED_NC = None


def _build_nc():
    """Build + compile the single-core program (same NEFF on all 8 cores)."""
    nc = bacc.Bacc("TRN2", target_bir_lowering=False, debug=False)
    f32, bf16 = dt.float32, dt.bfloat16
    Exp = mybir.ActivationFunctionType.Exp
    Copy = mybir.ActivationFunctionType.Copy
    mult = mybir.AluOpType.mult
    add = mybir.AluOpType.add

    Ident = mybir.ActivationFunctionType.Identity
    f8 = dt.float8e4
    DR = mybir.MatmulPerfMode.DoubleRow
    x8d, w8d = {}, {}
    for nm in ("xq", "xk", "xv"):
        for hl in "hl":
            x8d[nm + hl] = nc.dram_tensor(
                nm + hl, [D, T], f8, kind="ExternalInput").ap()
    for nm in ("wq", "wk", "wv"):
        for hl in "hl":
            w8d[nm + hl] = nc.dram_tensor(
                nm + hl, [D, 512], f8, kind="ExternalInput").ap()
    for nm in ("wo", "dt"):
        for hl in "hl":
            w8d[nm + hl] = nc.dram_tensor(
                nm + hl, [512, D] if nm == "wo" else [T, T], f8,
                kind="ExternalInput").ap()
    msk = nc.dram_tensor("msk", [P, 256], bf16, kind="ExternalInput").ap()
    bqc = nc.dram_tensor("bqc", [P, 4], f32, kind="ExternalInput").ap()
    bkc = nc.dram_tensor("bkc", [P, 4], f32, kind="ExternalInput").ap()
    bvr = nc.dram_tensor("bvr", [P, 512], bf16, kind="ExternalInput").ap()
    y = nc.dram_tensor("y", [T, D], bf16, kind="ExternalOutput").ap()

    with tile.TileContext(nc) as tc, ExitStack() as ctx:
        # ---- constants + persistent tiles ----------------------------------
        consts = ctx.enter_context(tc.tile_pool(name="consts", bufs=1))
        mskt = consts.tile([P, 256], bf16, tag="mskt")
        bqt = consts.tile([P, 4], f32, tag="bqt")
        bkt = consts.tile([P, 4], f32, tag="bkt")
        bvt = consts.tile([P, 512], bf16, tag="bvt")
        warm = consts.tile([P, 1], f32, tag="warm")

        persist = ctx.enter_context(tc.tile_pool(name="persist", bufs=1))
        qt = [persist.tile([P, T], bf16, tag=f"qt{p}", name=f"qt{p}") for p in range(4)]
        kt = [persist.tile([P, T], bf16, tag=f"kt{p}", name=f"kt{p}") for p in range(4)]
        # vt: per t-chunk [V_h | 1] blocks: col 65*h+e = V, col 65*h+64 = 1.0
        vt = [persist.tile([P, HG * (E + 1)], bf16, tag=f"vt{t}", name=f"vt{t}")
              for t in range(NTC)]
        # pvg: normalized softmax(S) @ V staging (heads side by side); only
        # feeds the fp8 hi/lo requantization now, so a ring of 8 suffices
        pvgr = [persist.tile([P, 512], bf16, tag=f"pvg{t}", name=f"pvg{t}")
                for t in range(8)]
        pvg = {j: pvgr[j % 8] for j in range(NTC)}
        # fp8 hi/lo requantization of pvg, packed in t-chunk PAIRS for the
        # DoubleRow D@PV in the tail; filled by DVE as pvg completes
        p8 = {hl: [persist.tile([P, 1024], dt.float8e4, tag=f"p8{hl}{tp}",
                                name=f"p8{hl}{tp}")
                   for tp in range(NTC // 2)] for hl in "hl"}

        # a2 accumulators for (qb=0, pc=0/1) — held across the attention loop
        # so part of D@PV streams in as attention fillers (2 psum banks).
        a2ps = ctx.enter_context(tc.tile_pool(name="a2ps", bufs=1, space="PSUM"))
        a2w = [a2ps.tile([P, 512], f32, tag=f"a2w{q}", name=f"a2w{q}")
               for q in range(2)]

        with tc.tile_pool(name="wpool", bufs=1) as wpool, \
             tc.tile_pool(name="xpool", bufs=1) as xpool, \
             tc.tile_pool(name="dtw", bufs=1) as dtwp, \
             tc.tile_pool(name="upool", bufs=1) as upool, \
             tc.tile_pool(name="rpool", bufs=4) as rpool, \
             tc.tile_pool(name="sps", bufs=2, space="PSUM") as sps, \
             tc.tile_pool(name="pvps", bufs=1, space="PSUM") as pvps, \
             tc.tile_pool(name="pjps", bufs=1, space="PSUM") as pjps:

            def load_wb(nm, split=False):
                tiles = []
                for hl in "hl":
                    wdram = w8d[nm + hl]
                    wb = wpool.tile([P, ND * 512], f8, tag=nm + hl, name=nm + hl)
                    nd = 2 if split else 1
                    hd = ND // nd
                    for s in range(nd):
                        nc.sync.dma_start(
                            wb[:, 512 * hd * s:512 * hd * (s + 1)].rearrange(
                                "p (d q) -> p d q", q=512),
                            wdram[P * hd * s:P * hd * (s + 1), :].rearrange(
                                "(d p) q -> p d q", p=P))
                    tiles.append(wb)
                    split = False
                return tiles  # [hi tile, lo tile]

            def drslice(tile_, kk, m0=None):
                """[128, 2 k-tiles, m] DoubleRow operand slice for super-chunk kk."""
                a = tile_[:, 1024 * kk:1024 * (kk + 1)].rearrange(
                    "p (j m) -> p j m", m=512)
                return a if m0 is None else a[:, :, m0:m0 + P]

            # wq + xq(0) load first: the prologue's first matmul needs them
            wq = load_wb("wq", split=True)

            # proj psum ring: one pjps bank, plus the two a2w banks which
            # are free until the D@PV fillers start accumulating at i=3
            pj0 = pjps.tile([P, 512], f32, tag="pj", name="pj")
            pjring = [[pj0, a2w[0], a2w[1]], [pj0]]
            pjslot = [0]

            # two generations of exp-score tiles (step parity) so PV of step
            # s-1 can interleave with scores of step s
            ut2 = [[upool.tile([P, 1024], bf16, tag=f"u{g}_{c}", name=f"u{g}_{c}")
                    for c in range(NTC)] for g in range(2)]
            mskr = mskt[:].rearrange("p (s c) -> p s c", c=P)
            pvt = pvps.tile([P, 390], f32, tag="pvt", name="pvt")  # 3 slots x 130
            pvslot = [0]

            def uchunk(par, c, h, jj):
                return ut2[par][c][:, 512 * h + P * jj:512 * h + P * (jj + 1)]

            xcur = {}

            def issue_x1(kind, tb, split=False):
                pair = []
                for hl in "hl":
                    dram = x8d[kind + hl]
                    xb = xpool.tile([P, ND * 512], f8, tag=kind + hl,
                                    name=kind + hl,
                                    bufs=2 if kind == "xk" else 1)
                    nd = 2 if split else 1
                    hd = ND // nd
                    src = dram[:, 512 * tb:512 * (tb + 1)]
                    for s in range(nd):
                        nc.sync.dma_start(
                            xb[:, 512 * hd * s:512 * hd * (s + 1)].rearrange(
                                "p (d q) -> p d q", q=512),
                            src[P * hd * s:P * hd * (s + 1), :].rearrange(
                                "(d p) q -> p d q", p=P))
                    pair.append(xb)
                    split = False
                xcur[(kind, tb)] = pair

            def issue_x(tb):
                for kind in ("xq", "xk", "xv"):
                    issue_x1(kind, tb)

            issue_x1("xq", 0, split=True)
            nc.sync.dma_start(bqt[:], bqc[:])
            wk = load_wb("wk")
            issue_x1("xk", 0)
            nc.sync.dma_start(bkt[:], bkc[:])
            wv = load_wb("wv")
            issue_x1("xv", 0)
            nc.sync.dma_start(bvt[:], bvr[:])
            nc.sync.dma_start(mskt[:], msk[:])
            nc.vector.memset(warm[:], 0.0)
            nc.scalar.activation(warm[:], warm[:], Exp)  # preload exp table

            dt8w = {}

            def issue_dt8w():
                for hl in "hl":
                    b = dtwp.tile([P, 12 * 512], f8, tag="dt8" + hl,
                                  name="dt8w")
                    nc.sync.dma_start(
                        b[:].rearrange("p (t q) -> p t q", q=512),
                        w8d["dt" + hl][0:1536, 0:512].rearrange(
                            "(t p) q -> p t q", p=P))
                    dt8w[hl] = b

            # ---- filler units (each emits ~0.8-2us of PE work) -------------
            # Projections run as fp8 DoubleRow 3-pass (hi*hi + hi*lo + lo*hi):
            # 12 matmuls of 256 k-depth each, 0.5 cycles/row.
            PASSES = ((0, 0), (0, 1), (1, 0))  # (stationary hi/lo, moving hi/lo)

            def proj_units(p, tb, which, proto=False):
                """Q or K projection of q-block tb for head pair p."""
                wt, bt, dest, kind = {
                    "q": (wq, bqt, qt, "xq"), "k": (wk, bkt, kt, "xk")}[which]

                def go(half, holder=[None]):
                    xp = xcur[(kind, tb)]
                    if half == 0:
                        if proto:  # borrow the idle scores ring (4 banks)
                            sp_ = sps.tile([P, 1024], f32, tag="sp", name="sp")
                            holder[0] = sp_[:, 0:512]
                        else:
                            # runs at block tb-1 (Q) / tb (K); a2w banks are
                            # free until the D@PV fillers start at block 3
                            blk = tb - 1 if which == "q" else tb
                            ring = pjring[0 if blk < 3 else 1]
                            holder[0] = ring[pjslot[0] % len(ring)]
                            pjslot[0] += 1
                    ps = holder[0]
                    for idx in range(6 * half, 6 * half + 6):
                        pw, px = PASSES[idx // 4]
                        kk = idx % 4
                        nc.tensor.matmul(
                            ps[:], drslice(wt[pw], kk, P * p),
                            drslice(xp[px], kk),
                            start=(idx == 0), stop=(idx == 11), perf_mode=DR)
                    if half == 1:
                        if proto or (which == "q" and tb <= 1):
                            # scalar engine is idle this early; DVE carries
                            # the PV normalize chains
                            nc.scalar.activation(
                                dest[p][:, 512 * tb:512 * (tb + 1)], ps[:],
                                Ident, bias=bt[:, p:p + 1])
                        else:
                            nc.vector.tensor_scalar(
                                dest[p][:, 512 * tb:512 * (tb + 1)], ps[:],
                                bt[:, p:p + 1], None, op0=add)
                return [lambda: go(0), lambda: go(1)]

            def v_proj_units(tb, tcc):
                """V projection for t-chunk 4*tb+tcc."""
                units = []

                def go(half, holder=[None]):
                    xp = xcur[("xv", tb)]
                    if half == 0:
                        ring = pjring[0 if tb < 3 else 1]
                        holder[0] = ring[pjslot[0] % len(ring)]
                        pjslot[0] += 1
                    ps = holder[0]
                    for idx in range(6 * half, 6 * half + 6):
                        px, pw = PASSES[idx // 4]
                        kk = idx % 4
                        nc.tensor.matmul(
                            ps[:], drslice(xp[px], kk, P * tcc),
                            drslice(wv[pw], kk),
                            start=(idx == 0), stop=(idx == 11), perf_mode=DR)
                    if half == 1:
                        t = 4 * tb + tcc
                        vtr = vt[t][:].rearrange("p (h c) -> p h c", c=E + 1)
                        psr = ps[:].rearrange("p (h c) -> p h c", c=E)
                        bvrr = bvt[:].rearrange("p (h c) -> p h c", c=E)
                        nc.vector.tensor_tensor(vtr[:, :, 0:E], psr, bvrr, op=add)
                        nc.vector.memset(vtr[:, :, E], 1.0)
                units.append(lambda: go(0))
                units.append(lambda: go(1))
                return units

            def dpv_unit(pc, pi):
                def go():
                    pa, pb = PASSES[pi]
                    sa, sb = "hl"[pa], "hl"[pb]
                    for tp in range(6):
                        lhsT = p8[sa][tp][:].rearrange(
                            "p (j m) -> p j m", m=512)[:, :, P * pc:P * (pc + 1)]
                        rhs = dt8w[sb][:, 1024 * tp:1024 * (tp + 1)].rearrange(
                            "p (j q) -> p j q", q=512)
                        nc.tensor.matmul(
                            a2w[pc][:], lhsT, rhs,
                            start=(pi == 0 and tp == 0), stop=False,
                            perf_mode=DR)
                return go

            # ---- prologue: Q/K projections for q-block 0 (all Q first so
            # the K weight/x DMAs hide behind Q compute) --------------------
            for p in range(4):
                for u_ in proj_units(p, 0, "q", proto=True):
                    u_()
            for p in range(4):
                for u_ in proj_units(p, 0, "k", proto=True):
                    u_()

            # ---- attention: flat pipeline over steps s = (i, p) ------------
            # scores of step s interleave with PV of step s-1 (exp stream on
            # the scalar engine never pauses) plus proj/D@PV fillers.
            def pv_group(ip, pp, par, jj):
                def go():
                    j = 4 * ip + jj
                    s0 = 130 * (pvslot[0] % 3)
                    pvslot[0] += 1
                    pvs = pvt[:, s0:s0 + 130]
                    for h in range(2):
                        for c in range(j + 1):
                            nc.tensor.matmul(
                                pvs[:, 65 * h:65 * (h + 1)],
                                uchunk(par, c, h, jj),
                                vt[c][:, 65 * (2 * pp + h):65 * (2 * pp + h + 1)],
                                start=(c == 0), stop=(c == j))
                    pvr = pvs.rearrange("p (h c) -> p h c", c=E + 1)
                    rcp = rpool.tile([P, 2], f32, tag="rcp", name="rcp")
                    nc.vector.reciprocal(rcp[:], pvr[:, :, E])
                    for h in range(2):
                        nc.vector.tensor_scalar(
                            pvg[j][:, E * (2 * pp + h):E * (2 * pp + h + 1)],
                            pvs[:, 65 * h:65 * h + E],
                            rcp[:, h:h + 1], None, op0=mult)
                    if pp == 3:  # pvg[j] complete: fp8 hi/lo for the DR
                        # tail. GpSimd (SBUF-only, mostly idle); the last
                        # four gate the tail start, so those go on DVE
                        eng = nc.vector if j >= 12 else nc.gpsimd
                        s8 = slice(512 * (j % 2), 512 * (j % 2) + 512)
                        eng.tensor_copy(p8["h"][j // 2][:, s8], pvg[j][:])
                        eng.tensor_tensor(
                            p8["l"][j // 2][:, s8], pvg[j][:],
                            p8["h"][j // 2][:, s8], op=mybir.AluOpType.subtract)
                return go

            step = 0
            pvq = []
            for i in range(4):
                if i < 3:
                    issue_x(i + 1)
                fillers = []
                for tcc in range(4):
                    fillers += v_proj_units(i, tcc)
                if i == 3:
                    issue_dt8w()
                    for pc in range(2):
                        for pi in range(3):
                            fillers.append(dpv_unit(pc, pi))
                if i < 3:
                    for p in range(4):
                        fillers += proj_units(p, i + 1, "q")
                # K projections of THIS block are emitted lazily inside each
                # step, right before its first diagonal score chunk
                kunits = {p: (proj_units(p, i, "k") if i >= 1 else [])
                          for p in range(4)}
                fp = [0]

                def pump(k, fillers=fillers, fp=fp):
                    while k > 0 and fp[0] < len(fillers):
                        fillers[fp[0]]()
                        fp[0] += 1
                        k -= 1

                def score_chunk(ii, pp, par, c):
                    j0 = max(0, c - 4 * ii)
                    sp = sps.tile([P, 1024], f32, tag="sp", name="sp")
                    for h in range(2):
                        nc.tensor.matmul(
                            sp[:, 512 * h + P * j0:512 * (h + 1)],
                            kt[pp][64 * h:64 * (h + 1), P * c:P * (c + 1)],
                            qt[pp][64 * h:64 * (h + 1),
                                   512 * ii + P * j0:512 * (ii + 1)],
                            start=True, stop=True)
                    spr = sp[:].rearrange("p (h w) -> p h w", w=512)
                    ur = ut2[par][c][:].rearrange("p (h w) -> p h w", w=512)
                    nc.scalar.activation(
                        ur[:, :, P * j0:512], spr[:, :, P * j0:512],
                        Exp, scale=COEF / (SCL * SCL))
                    if c >= 4 * ii:
                        # mask on GpSimd: SBUF-only, engine otherwise idle
                        nc.gpsimd.tensor_tensor(
                            ur[:, :, P * j0:P * (j0 + 1)],
                            ur[:, :, P * j0:P * (j0 + 1)], mskr, op=mult)

                nch = 4 * (i + 1)
                for p in range(4):
                    par = step % 2
                    c0 = PRE3 if (i == 3 and p == 0) else 0
                    for c in range(c0, nch):
                        if c == c0 + 1:  # K proj of this block before diag use
                            for u_ in kunits[p]:
                                u_()
                            kunits[p] = []
                        score_chunk(i, p, par, c)
                        if c % 2 == 0 and pvq:
                            pvq.pop(0)()
                        elif c % 2 == 1:
                            pump(1)
                    while pvq:
                        pvq.pop(0)()
                    if p == 0:
                        pump(8)  # all V-proj of this i before its PV starts
                    pump((len(fillers) * (p + 1)) // 4 - fp[0])
                    pvq = [pv_group(i, p, par, jj) for jj in range(4)]
                    step += 1
                if i == 2:
                    # pre-compute the first exp chunks of block 3's first
                    # step in block 2's scalar-engine slack (par of step 12
                    # is 0; its gen-0 u tiles are free by now), interleaved
                    # with the pending PV groups of step (2,3) and leftover
                    # fillers so the exp pipeline stalls stay absorbed
                    for c in range(PRE3):
                        score_chunk(3, 0, 0, c)
                        if c % 2 == 0 and pvq:
                            pvq.pop(0)()
                        elif c % 2 == 1:
                            pump(1)
                pump(len(fillers))
            while pvq:
                pvq.pop(0)()

        # ---- tail: finish D@PV + output projection, both fp8 DoubleRow -----
        with tc.tile_pool(name="dtp", bufs=3) as dtp, \
             tc.tile_pool(name="wo", bufs=1) as wop, \
             tc.tile_pool(name="a2s", bufs=1) as a2sp, \
             tc.tile_pool(name="obuf", bufs=3) as obp, \
             tc.tile_pool(name="aps", bufs=1, space="PSUM") as aps, \
             tc.tile_pool(name="ops", bufs=2, space="PSUM") as ops:
            wo8 = {}
            for hl in "hl":
                w = wop.tile([P, 4 * D], f8, tag="wo" + hl, name="wo" + hl)
                nc.sync.dma_start(
                    w[:].rearrange("p (c q) -> p c q", q=D),
                    w8d["wo" + hl][:].rearrange("(c p) q -> p c q", p=P))
                wo8[hl] = w

            dtb8 = {}

            def issue_dtb8(qb):
                pr = {}
                for hl in "hl":
                    b = dtp.tile([P, NTC * 512], f8, tag="dtb" + hl,
                                 name="dtb" + hl)
                    nc.sync.dma_start(
                        b[:].rearrange("p (t q) -> p t q", q=512),
                        w8d["dt" + hl][:, 512 * qb:512 * (qb + 1)].rearrange(
                            "(t p) q -> p t q", p=P))
                    pr[hl] = b
                dtb8[qb] = pr

            a2s8 = {}

            def a2s_copy(qb, pc, src):
                if qb not in a2s8:
                    a2s8[qb] = {hl: a2sp.tile([P, 2048], f8,
                                              tag=f"a2s{hl}{qb}",
                                              name=f"a2s{hl}{qb}")
                                for hl in "hl"}
                dh = a2s8[qb]["h"][:, 512 * pc:512 * (pc + 1)]
                dl = a2s8[qb]["l"][:, 512 * pc:512 * (pc + 1)]
                nc.scalar.activation(dh, src[:], Copy)
                nc.vector.tensor_tensor(
                    dl, src[:], dh, op=mybir.AluOpType.subtract)

            def dpv_tail(qb, pcs):
                a2 = {pc: aps.tile([P, 512], f32, tag=f"a2t{pc}", name="a2t")
                      for pc in pcs}
                for pc in pcs:
                    n = 0
                    for pa, pb in PASSES:
                        sa, sb = "hl"[pa], "hl"[pb]
                        for tp in range(8):
                            lhsT = p8[sa][tp][:].rearrange(
                                "p (j m) -> p j m", m=512)[:, :, P * pc:P * (pc + 1)]
                            rhs = dtb8[qb][sb][:, 1024 * tp:1024 * (tp + 1)].rearrange(
                                "p (j q) -> p j q", q=512)
                            nc.tensor.matmul(
                                a2[pc][:], lhsT, rhs,
                                start=(n == 0), stop=(n == 23), perf_mode=DR)
                            n += 1
                    a2s_copy(qb, pc, a2[pc])

            def out_proj(qb, qss=range(4)):
                for qs in qss:
                    ob = obp.tile([P, D], bf16, tag="ob", name="ob")
                    for nh in range(2):
                        op_ = ops.tile([P, 512], f32, tag="op", name="op")
                        n = 0
                        for pa, pb in PASSES:
                            sa, sb = "hl"[pa], "hl"[pb]
                            for kk in range(2):
                                lhsT = a2s8[qb][sa][:, 1024 * kk:1024 * (kk + 1)
                                                    ].rearrange(
                                    "p (j m) -> p j m", m=512)[
                                    :, :, P * qs:P * (qs + 1)]
                                rhs = wo8[sb][:, 2048 * kk:2048 * (kk + 1)
                                              ].rearrange(
                                    "p (j q) -> p j q", q=D)[
                                    :, :, 512 * nh:512 * (nh + 1)]
                                nc.tensor.matmul(
                                    op_[:], lhsT, rhs,
                                    start=(n == 0), stop=(n == 5), perf_mode=DR)
                                n += 1
                        if qb < 2:
                            nc.scalar.activation(
                                ob[:, 512 * nh:512 * (nh + 1)], op_[:], Copy,
                                scale=1.0 / 8192.0)
                        else:
                            nc.vector.tensor_scalar(
                                ob[:, 512 * nh:512 * (nh + 1)], op_[:],
                                1.0 / 8192.0, None, op0=mult)
                        nc.sync.dma_start(
                            y[512 * qb + P * qs:512 * qb + P * (qs + 1),
                              512 * nh:512 * (nh + 1)],
                            ob[:, 512 * nh:512 * (nh + 1)])

            issue_dtb8(0)
            # finish the in-window accumulators (qb0, pc 0/1): t = 12..15
            for pc in range(2):
                n = 0
                for pa, pb in PASSES:
                    sa, sb = "hl"[pa], "hl"[pb]
                    for tp in (6, 7):
                        lhsT = p8[sa][tp][:].rearrange(
                            "p (j m) -> p j m", m=512)[:, :, P * pc:P * (pc + 1)]
                        rhs = dtb8[0][sb][:, 1024 * tp:1024 * (tp + 1)
                                          ].rearrange("p (j q) -> p j q", q=512)
                        nc.tensor.matmul(
                            a2w[pc][:], lhsT, rhs,
                            start=False, stop=(n == 5), perf_mode=DR)
                        n += 1
            for pc in range(2):
                a2s_copy(0, pc, a2w[pc])
            dpv_tail(0, [2, 3])
            issue_dtb8(1)
            dpv_tail(1, [0, 1, 2, 3])
            out_proj(0)
            issue_dtb8(2)
            dpv_tail(2, [0, 1, 2, 3])
            out_proj(1)
            issue_dtb8(3)
            dpv_tail(3, [0, 1])
            out_proj(2, (0, 1))
            dpv_tail(3, [2, 3])
            out_proj(2, (2, 3))
            out_proj(3)

    nc.compile()
    return nc


F8NP = ml_dtypes.float8_e4m3


def _f8split(a):
    """(hi, lo) fp8 e4m3 split: hi + lo ~ a with ~0.1% relative error."""
    hi = np.asarray(a, F8NP)
    lo = np.asarray(a - hi.astype(np.float32), F8NP)
    return hi, lo


def _prep_inputs(query_1, key_1, value_1, Wq, bq, Wk, bk, Wv, bv, Wo, bo, Dmat):
    """Host-side sharding + fp8/bf16 packing: per-core input dicts."""
    f = np.float32

    def xT(x, b):
        return _f8split(np.ascontiguousarray(np.asarray(x[b], f).T))

    per_g = []
    for g in range(2):
        h0 = HG * g
        wqg = np.zeros((D, 512), f)
        wkg = np.zeros((D, 512), f)
        bqg = np.zeros((P, 4), f)
        bkg = np.zeros((P, 4), f)
        for p in range(4):
            for h in range(2):
                hh = h0 + 2 * p + h
                c0 = 128 * p + 64 * h
                wqg[:, c0:c0 + 64] = np.asarray(Wq[hh], f).T * SCL
                wkg[:, c0:c0 + 64] = np.asarray(Wk[hh], f).T * SCL
                bqg[64 * h:64 * (h + 1), p] = np.asarray(bq[hh], f) * SCL
                bkg[64 * h:64 * (h + 1), p] = np.asarray(bk[hh], f) * SCL
        wvg = np.zeros((D, 512), f)
        bvg = np.zeros((512,), f)
        for j in range(HG):
            wvg[:, 64 * j:64 * (j + 1)] = np.asarray(Wv[h0 + j], f).T * SCL
            bvg[64 * j:64 * (j + 1)] = np.asarray(bv[h0 + j], f) * SCL
        wog = np.ascontiguousarray(
            np.asarray(Wo, f)[:, 64 * h0:64 * h0 + 512].T) * SCL
        gd = {}
        gd["wqh"], gd["wql"] = _f8split(wqg)
        gd["wkh"], gd["wkl"] = _f8split(wkg)
        gd["wvh"], gd["wvl"] = _f8split(wvg)
        gd["woh"], gd["wol"] = _f8split(wog)
        gd["bqc"], gd["bkc"] = bqg, bkg
        gd["bvr"] = np.broadcast_to(bvg, (P, 512)).astype(BF)
        per_g.append(gd)

    dTs = np.ascontiguousarray(np.asarray(Dmat, f).T) * SCLD
    dth, dtl_ = _f8split(dTs)
    m = (np.arange(128)[None, :] >= np.arange(128)[:, None]).astype(f)  # [k, q]
    msk2 = np.ascontiguousarray(np.tile(m, (1, 2))).astype(BF)

    xs = {}
    for nm, src in (("xq", query_1), ("xk", key_1), ("xv", value_1)):
        for b in range(B):
            xs[(nm, b, "h")], xs[(nm, b, "l")] = xT(src, b)

    in_maps = []
    for c in range(8):
        b, g = c // 2, c % 2
        im = {"dth": dth, "dtl": dtl_, "msk": msk2}
        im.update(per_g[g])
        for nm in ("xq", "xk", "xv"):
            im[nm + "h"] = xs[(nm, b, "h")]
            im[nm + "l"] = xs[(nm, b, "l")]
        in_maps.append(im)
    return in_maps


def kernel(query_1, key_1, value_1, Wq, bq, Wk, bk, Wv, bv, Wo, bo, D):
    import os
    os.environ["BASS_NEVER_TRACE"] = "1"  # NTFF capture hangs over the axon relay
    global _CACHED_NC
    if _CACHED_NC is None:
        _CACHED_NC = _build_nc()
    nc = _CACHED_NC
    in_maps = _prep_inputs(query_1, key_1, value_1, Wq, bq, Wk, bk, Wv, bv, Wo, bo, D)
    res = run_bass_kernel_spmd(nc, in_maps, core_ids=list(range(8)))
    bo_f = np.asarray(bo, np.float32)
    out = np.empty((B, T, 1024), np.float32)
    for b in range(B):
        out[b] = (res.results[2 * b]["y"].astype(np.float32)
                  + res.results[2 * b + 1]["y"].astype(np.float32) + bo_f)
    return out
